# revision 1
# baseline (speedup 1.0000x reference)
"""Self-contained Trainium2 kernel: block-circulant FFT linear layer.

out = ifft(fft(x_blocks) * fft(W)).real summed over input blocks -- computed
as three PE matmul stages (real-FFT basis, per-frequency block matmul,
inverse real-FFT) with DVE 32x32 block-transposes as the inter-stage data
shuffles, SPMD over 8 NeuronCores (batch-sharded).

kernel(x, W): x [4096, 4096] f32, W [64, 64, 64] f32 -> [4096, 4096] f32.
"""
import numpy as np
import concourse.bass as bass
import concourse.bacc as bacc
import concourse.mybir as mybir
import concourse.tile as tile
from concourse.tile import add_dep_helper
from concourse.bass_utils import run_bass_kernel_spmd

N_CORES = 8
B, IN, OUT, BS = 4096, 4096, 4096, 64
BC = B // N_CORES            # 512 batch rows per core
NK = 32                      # bin tiles (tile 0 carries bins 0 and 32)
NA = 32                      # jpair / ipair tiles
F32 = mybir.dt.float32
F32R = mybir.dt.float32r
MM_DT = F32R   # matmul input dtype (float32r: 4x PE rate, rounded inputs)

# tunables
BW = 256                     # batch window (matmul free dim)
NH = BC // BW
IN_SPLIT = 2                 # HBM in/out DMAs per window
SHUF_MODE = "strided"
USE_BARRIER = False
LEVEL = 5  # 0:io 1:+fwd 2:+sh1 3:+mid 4:+sh2 5:+inv
SH_ENGINES = ("alt", "alt")     # "alt": even idx sync, odd idx scalar
IO_ENGINES = ("sync", "scalar")   # engines for (in, out) DMA issue


# ---------------- host-side constant matrices ----------------

def make_fmat():
    t = np.arange(BS)[:, None]
    c = np.arange(BS)[None, :]
    k = np.where(c <= 32, c, c - 32)
    ang = 2 * np.pi * k * t / BS
    F = np.where(c <= 32, np.cos(ang), np.sin(ang))
    bd = np.zeros((128, 128), np.float32)
    bd[:64, :64] = F
    bd[64:, 64:] = F
    return bd


def make_gmat():
    tau = np.arange(BS)[None, :]
    c = np.arange(BS)[:, None]
    k = np.where(c <= 32, c, c - 32)
    ang = 2 * np.pi * k * tau / BS
    base = np.where(c <= 32, np.cos(ang), np.sin(ang))
    scale = np.where((c % 32) == 0, 1.0 / BS, 2.0 / BS)
    G = base * scale
    bd = np.zeros((128, 128), np.float32)
    bd[:64, :64] = G
    bd[64:, 64:] = G
    return bd


def make_wmats(W):
    s = np.arange(BS)
    k = np.arange(33)
    ang = 2 * np.pi * k[:, None] * s[None, :] / BS
    wr = np.einsum("ijs,ks->ijk", W, np.cos(ang))
    wi = np.einsum("ijs,ks->ijk", W, np.sin(ang))
    M = np.zeros((NK, 128, 128), np.float32)

    def colperm(Wblk):
        # [i, j] -> [row j-perm, col i-perm]: cols 32*(2*par) + a ordering
        # returns [64 j, 64 i-col] for one c2 quadrant pair handled below
        return Wblk

    # row order r: 0..31 even-j Re, 32..63 even-j Im, 64..95 odd-j Re, 96..127 odd-j Im
    # col order m: 32*(2*par + c2) + a  for i = 2a+par, c2 in {Re:0, Im:1}
    icol = np.empty(64, np.int64)  # icol[i-block] base col group by parity
    for i in range(64):
        a, par = divmod(i, 2)
        icol[i] = 64 * par + a  # Re col for block i; Im col = +32
    for kk in range(NK):
        if kk == 0:
            WrE, WiE = wr[:, :, 0], None   # bins 0 / 32 packed
            W32 = wr[:, :, 32]
        Wr, Wi = wr[:, :, kk], wi[:, :, kk]
        for par_j in range(2):
            jrows = np.arange(32) * 2 + par_j       # j block index
            rre = 64 * par_j + np.arange(32)        # row for (a_j, par_j, Re)
            rim = rre + 32
            for i in range(64):
                cre = icol[i]
                cim = cre + 32
                if kk == 0:
                    M[0, rre, cre] = wr[i, jrows, 0]
                    M[0, rim, cim] = W32[i, jrows]
                else:
                    M[kk, rre, cre] = Wr[i, jrows]
                    M[kk, rim, cre] = -Wi[i, jrows]
                    M[kk, rre, cim] = Wi[i, jrows]
                    M[kk, rim, cim] = Wr[i, jrows]
    # device layout: [row r, tile k, col m]
    return np.ascontiguousarray(M.transpose(1, 0, 2))


def prep_x(x):
    """[B, 4096] -> per-core [NH, 128, 32, BW]; p = par*64+t, j = 2a+par."""
    xr = x.reshape(N_CORES, NH, BW, 32, 2, 64)  # [c, h, w, a, par, t]
    xp = np.ascontiguousarray(xr.transpose(0, 1, 4, 5, 3, 2))  # [c,h,par,t,a,w]
    return xp.reshape(N_CORES, NH, 128, 32, BW)


def post_y(ys):
    """per-core [NH, 128, BW, NA] -> [B, 4096]; p = par*64 + tau, i = 2a+par."""
    y = np.stack(ys)  # [c, NH, 128, BW, NA]
    y = y.reshape(N_CORES, NH, 2, 64, BW, NA)  # [c,h,par,tau,w,a]
    y = y.transpose(0, 1, 4, 5, 2, 3)  # [c,h,w,a,par,tau]
    return np.ascontiguousarray(y).reshape(B, OUT)


# ---------------- device kernel ----------------

def evac_engine(idx):
    # split PSUM evacuation between DVE and ACT (DVE also does transposes)
    return "vector" if idx % 3 == 2 else "scalar"


def _evac(nc, eng, dst, src):
    if eng == "vector":
        return nc.vector.tensor_copy(dst, src)
    return nc.scalar.copy(dst, src)


def build_nc(reps=1):
    """v3: shuffles via DVE StreamTranspose (32x32 block transposes).

    Spectral tiles live in (w-major, comp-inner) free layout:
      s_sb [128, BW, NA]: f = w*32 + a   (comp rows, per-jpair chunks)
      s2   [128, BW, NK]: f = w*32 + k   (bin-major rows after transpose)
      o_sb [128, BW, NK]: f = w*32 + k
      v    [128, BW, NA]: f = w*32 + a
    DVE block-transpose swaps (a<->row-within-quarter) per 32x32 block.
    """
    nc = bacc.Bacc("TRN2", target_bir_lowering=False, debug=False,
                   num_devices=N_CORES, dynamic_dma_scratch_size=8192)
    x_in = nc.dram_tensor("x", [NH, 128, NA, BW], MM_DT, kind="ExternalInput")
    fmat = nc.dram_tensor("fmat", [128, 128], MM_DT, kind="ExternalInput")
    gmat = nc.dram_tensor("gmat", [128, 128], F32, kind="ExternalInput")
    wmat = nc.dram_tensor("wmat", [128, NK, 128], F32, kind="ExternalInput")
    y_out = nc.dram_tensor("y", [NH, 128, BW, NA], F32, kind="ExternalOutput")

    ASPL = NA // IN_SPLIT

    with tile.TileContext(nc) as tc:
        with (
            tc.tile_pool(name="consts", bufs=1) as cpool,
            tc.tile_pool(name="p1", bufs=2) as p1,   # xw / o_sb
            tc.tile_pool(name="p2", bufs=2) as p2,   # s_sb / v
            tc.tile_pool(name="p3", bufs=2) as p3,   # s2 / y
            tc.tile_pool(name="fps", bufs=3, space="PSUM") as fps,
            tc.tile_pool(name="mps", bufs=2, space="PSUM") as mps,
            tc.tile_pool(name="ips", bufs=3, space="PSUM") as ips,
        ):
            f_sb = cpool.tile([128, 128], MM_DT)
            g_sb = cpool.tile([128, 128], F32)
            w_sb = cpool.tile([128, NK, 128], F32)
            nc.sync.dma_start(f_sb[:], fmat[:])
            nc.sync.dma_start(g_sb[:], gmat[:])
            nc.sync.dma_start(w_sb[:], wmat[:])

            for _ in range(reps):
                for h in range(NH):
                    xw = p1.tile([128, NA, BW], MM_DT, tag="a")
                    for s in range(IN_SPLIT):
                        getattr(nc, IO_ENGINES[0]).dma_start(
                            xw[:, s * ASPL:(s + 1) * ASPL, :],
                            x_in[h, :, s * ASPL:(s + 1) * ASPL, :])
                    last = xw

                    # FWD: out columns ordered (w, a2) to match s_sb layout
                    s_sb = p2.tile([128, BW, NA], F32, tag="b")
                    for a in [] if LEVEL < 1 else range(0, NA, 2):
                        ps = fps.tile([128, BW, 2], F32, tag="fps")
                        rhs = xw[:, a:a + 2, :].rearrange("p a w -> p w a")
                        nc.tensor.matmul(ps[:], f_sb[:], rhs)
                        _evac(nc, evac_engine(a // 2), s_sb[:, :, a:a + 2],
                              ps[:])
                    if LEVEL >= 1:
                        last = s_sb

                    s2 = p3.tile([128, BW, NK], F32, tag="c")
                    if LEVEL >= 2:
                        nc.vector.transpose(s2[:], s_sb[:])
                        last = s2

                    o_sb = p1.tile([128, BW, NK], F32, tag="a")
                    for k in [] if LEVEL < 3 else range(NK):
                        ps = mps.tile([128, BW], F32, tag="mps")
                        nc.tensor.matmul(ps[:], w_sb[:, k, :], s2[:, :, k])
                        _evac(nc, evac_engine(k + 1), o_sb[:, :, k], ps[:])
                    if LEVEL >= 3:
                        last = o_sb

                    v_sb = p2.tile([128, BW, NA], F32, tag="b")
                    if LEVEL >= 4:
                        nc.vector.transpose(v_sb[:], o_sb[:])
                        last = v_sb

                    y_sb = p3.tile([128, BW, NA], F32, tag="c")
                    for a in [] if LEVEL < 5 else range(0, NA, 2):
                        ps = ips.tile([128, BW, 2], F32, tag="ips")
                        nc.tensor.matmul(ps[:], g_sb[:], v_sb[:, :, a:a + 2])
                        _evac(nc, evac_engine(a // 2 + 2),
                              y_sb[:, :, a:a + 2], ps[:])

                    out_src = y_sb if LEVEL >= 5 else last
                    WSPL = BW // IN_SPLIT
                    for s in range(IN_SPLIT):
                        if LEVEL >= 5:
                            getattr(nc, IO_ENGINES[1]).dma_start(
                                y_out[h, :, s * WSPL:(s + 1) * WSPL, :],
                                y_sb[:, s * WSPL:(s + 1) * WSPL, :])
                        else:
                            getattr(nc, IO_ENGINES[1]).dma_start(
                                y_out[h].rearrange("p w c -> p (w c)")[
                                    :, s * (NA * BW // IN_SPLIT):
                                    (s + 1) * (NA * BW // IN_SPLIT)],
                                out_src[:].rearrange(
                                    "p w c -> p (w c)" if last is not xw
                                    else "p c w -> p (c w)")[
                                    :, s * (NA * BW // IN_SPLIT):
                                    (s + 1) * (NA * BW // IN_SPLIT)])

    nc.compile()
    return nc


_NC_CACHE = {}


def run(x, W, reps=1):
    if reps not in _NC_CACHE:
        _NC_CACHE[reps] = build_nc(reps)
    nc = _NC_CACHE[reps]
    fmat = make_fmat()
    gmat = make_gmat()
    wmat = make_wmats(np.asarray(W, np.float32))
    xp = prep_x(np.ascontiguousarray(np.asarray(x, np.float32)))
    in_maps = [
        {"x": xp[c], "fmat": fmat, "gmat": gmat, "wmat": wmat}
        for c in range(N_CORES)
    ]
    res = run_bass_kernel_spmd(nc, in_maps, list(range(N_CORES)))
    return post_y([res.results[c]["y"] for c in range(N_CORES)])




_NC = None


def kernel(x, W):
    global _NC
    if _NC is None:
        _NC = build_nc(reps=1)
    fmat = make_fmat()
    gmat = make_gmat()
    wmat = make_wmats(np.asarray(W, np.float32))
    xp = prep_x(np.ascontiguousarray(np.asarray(x, np.float32)))
    in_maps = [
        {"x": xp[c], "fmat": fmat, "gmat": gmat, "wmat": wmat}
        for c in range(N_CORES)
    ]
    res = run_bass_kernel_spmd(nc=_NC, in_maps=in_maps,
                               core_ids=list(range(N_CORES)))
    return post_y([res.results[c]["y"] for c in range(N_CORES)])

